# revision 21
# baseline (speedup 1.0000x reference)
"""Trainium2 Bass kernel for nn_GNN_82781199663565 (gnn_message_passing).

Computation (see reference):
  du = relu(BN(einsum(h_att[1]*xp, Wu)))   # [B, 40, H, W]
  dl = relu(BN(einsum(h_att[2]*xp, Wl)))   # [B, 20, H, W]
  p_new[0]   = 0.5*(h_nodes[0] + p_nodes[0])
  p_new[1:5] = 0.5*(p_nodes[1:5] + du4)    # du reshaped to [4, B, 10, H, W]
  p_new[5:7] = 0.5*(p_nodes[5:7] + dl2)
(f_nodes, h_att[0], h_nodes[1:] are unused.)

Strategy vs the fp32 baseline (155us):
 - All bulk tensors cast to bf16 on host: xp stream 16.8->8.4 MB/core,
   p_nodes prefetch + output store halved.  Matmuls run bf16 (4x PE rate).
 - The [128, n] replicated attention operand is no longer streamed from
   HBM (4.2 MB/core); instead the compact [4, n] h_att rows are broadcast
   across partitions on the PE via a 0/1 selector matmul (L4), and applied
   to z from PSUM.
 - Sync-BN stats exchange: instead of the ncfw AllReduce (~28us end to
   end) each core writes its [128,4] partial-sums tile directly into the
   SBUF of all 7 peers with XOR-relative remote_dma_broadcast (slot j of
   my gather buffer is written by core me^j), then reduces locally on DVE.
 - Cross-batch stat folding uses a swapped-halves copy of the stats made
   with one SBUF->SBUF DMA, so the entire BN fold is small DVE ops - no
   tensor-engine matmuls on the post-collective critical path.
 - p_nodes are pre-scaled by 0.5 on the host, so phase 3 is
   d = relu(s*y + t) (ACT) ; out = d + pn (DVE), tiles split across
   engines.
"""
import sys
sys.path.insert(0, '/opt/trn_rl_repo')

import numpy as np
import ml_dtypes

BF16 = np.dtype(ml_dtypes.bfloat16)

N_CORES = 8
B, C, HID, H, W = 2, 256, 10, 256, 256
EPS = 1e-5
HS = H // N_CORES            # 32 H-rows per core
SPB = HS * W                 # spatial elems per batch image per core: 8192
M = 60                       # real output channels (40 u + 20 l)
MP = 64                      # padded to 64 -> groups tile partitions exactly
PP = 128
NW = 1024                    # window columns for vector/scalar ops
XN = 4096                    # xp super-tile columns (1 MiB bf16 DMAs)
QS = SPB // XN               # 2 super-iterations
WPS = XN // NW               # 4 windows per super-iteration
NWT = SPB // NW              # 8 windows total
NTOT = float(B * H * W)      # BN stat count: 131072

# NOTE: a direct remote_dma_broadcast stats exchange (XOR-relative SPMD
# all-gather, ~4us) was designed and built, but this container's walrus
# compiler rejects the remote-DMA ucode ISA ("ISA wrong length"), so the
# ncfw AllReduce (~35-45us exposed: entry barrier + trigger poll + mesh)
# is the only available cross-core primitive.

_built = None


def _build():
    import concourse.bass as bass
    import concourse.tile as tile
    from concourse import mybir
    from concourse.bass import _add_dep_helper
    import bass_rust

    f32 = mybir.dt.float32
    bf16 = mybir.dt.bfloat16
    Alu = mybir.AluOpType
    Act = mybir.ActivationFunctionType

    nc = bass.Bass("TRN2", target_bir_lowering=False, debug=False,
                   num_devices=N_CORES)

    xp_d = nc.dram_tensor("xp", [C, B * SPB], bf16, kind="ExternalInput").ap()
    att4_d = nc.dram_tensor("att4", [4, SPB], bf16, kind="ExternalInput").ap()
    pn_d = nc.dram_tensor("pn", [PP, SPB], bf16, kind="ExternalInput").ap()
    pn0_d = nc.dram_tensor("pn0", [128, 1280], bf16, kind="ExternalInput").ap()
    hn0_d = nc.dram_tensor("hn0", [128, 1280], bf16, kind="ExternalInput").ap()
    wp_d = nc.dram_tensor("wpack", [128, 512], bf16, kind="ExternalInput").ap()
    l4_d = nc.dram_tensor("l4", [4, 128], bf16, kind="ExternalInput").ap()
    cpk_d = nc.dram_tensor("cpk", [128, 2], f32, kind="ExternalInput").ap()

    out_d = nc.dram_tensor("out_main", [PP, SPB], bf16,
                           kind="ExternalOutput").ap()
    out0_d = nc.dram_tensor("out0", [128, 1280], bf16,
                            kind="ExternalOutput").ap()

    def pe_anchor(psum_tile, wp):
        # tiny matmul writing one psum element: absorbs the psum slot-release
        # wait so real matmuls carry <=1 wait
        nc.tensor.matmul(psum_tile[0:1, 0:1], wp[0:1, 0:1], wp[0:1, 0:1],
                         start=True, stop=True, skip_group_check=True)

    with tile.TileContext(nc) as tc:
        with (
            tc.tile_pool(name="consts", bufs=1) as cpool,
            tc.tile_pool(name="attc", bufs=1) as attc,
            tc.tile_pool(name="xin", bufs=2) as xin,
            tc.tile_pool(name="attsb", bufs=3) as attsb,
            tc.tile_pool(name="ybuf", bufs=1) as ybuf,
            tc.tile_pool(name="sqs", bufs=2) as sqs,
            tc.tile_pool(name="pnl", bufs=1) as pnl,
            tc.tile_pool(name="p0l", bufs=1) as p0l,
            tc.tile_pool(name="sm", bufs=1) as sm,
            tc.tile_pool(name="dbuf", bufs=3) as dbuf,
            # PSUM banking: z gets 3 buffers (6 banks) so the LAST window's
            # matmuls are not gated on the stt of window k-2 - that wait sat
            # directly on the serial tail that fires the stats trigger.
            # att drops to 1 buffer (2 banks): its copy (1.2us) hides under
            # each window's ~2.3us of z-matmul work.
            tc.tile_pool(name="zp", bufs=3, space="PSUM") as zp,
            tc.tile_pool(name="app", bufs=1, space="PSUM") as app,
            tc.tile_pool(name="dram", bufs=1, space="DRAM") as dr,
        ):
            wp = cpool.tile([128, 512], bf16)
            nc.sync.dma_start(wp[:], wp_d[:])
            l4t = cpool.tile([4, 128], bf16)
            nc.sync.dma_start(l4t[:], l4_d[:])
            cpk = cpool.tile([128, 2], f32)
            nc.sync.dma_start(cpk[:], cpk_d[:])
            gam = cpk[:, 0:1]                    # 0.5*gamma (z layout)
            bet = cpk[:, 1:2]                    # 0.5*beta
            wt = [wp[:, 0:MP], wp[:, MP:2 * MP]]

            att4t = attc.tile([4, SPB], bf16)
            nc.sync.dma_start(att4t[:], att4_d[:])

            y_full = ybuf.tile([PP, SPB], bf16, tag="y")
            out_full = ybuf.tile([PP, SPB], bf16, tag="o")
            s1t = sm.tile([PP, NWT + 1], f32, tag="s1t")
            s2t = sm.tile([PP, NWT + 1], f32, tag="s2t")

            # ---- PE warm-up: ~3us of dummy matmuls trips the HAM into
            # the 2.4 GHz state before the first xp tile lands ----
            wz = zp.tile([PP, NW], f32, tag="z", name="warm_z")
            for _ in range(8):
                nc.tensor.matmul(wz[0:MP, 0:384], wp[:, 0:MP], wp[:, 128:512],
                                 start=True, stop=True, skip_group_check=True)

            # broadcast the LAST window's attention early: removes the PSUM
            # copy from the post-stream serial tail that gates the stats
            # trigger
            ap7 = app.tile([PP, NW], f32, tag="att", name="att_pre7")
            pe_anchor(ap7, wp)
            for h in range(2):
                gs = slice(SPB - NW + h * 512, SPB - NW + (h + 1) * 512)
                nc.tensor.matmul(ap7[:, h * 512:(h + 1) * 512],
                                 l4t[:], att4t[:, gs], start=True, stop=True)
            asb7 = attsb.tile([PP, NW], bf16, tag="asb7", bufs=1, name="asb7")
            nc.scalar.copy(asb7[:], ap7[:])

            # ---- phase 1: stream xp, matmul, att broadcast, y = z*a ----
            xq = {}
            pnt = {}
            last_x = None
            for qs in range(QS):
                for b in range(B):
                    for c in range(2):
                        t = xin.tile([128, XN], bf16, tag=f"x{b}{c}",
                                     name=f"x{b}{c}_{qs}")
                        lo = b * SPB + qs * XN
                        if qs == 0 and b == 0 and c == 0:
                            # split first load: matmuls start on the first
                            # half while the second half streams
                            nc.sync.dma_start(
                                t[:, 0:XN // 2],
                                xp_d[c * 128:(c + 1) * 128, lo:lo + XN // 2])
                            xd = nc.sync.dma_start(
                                t[:, XN // 2:XN],
                                xp_d[c * 128:(c + 1) * 128,
                                     lo + XN // 2:lo + XN])
                        else:
                            xd = nc.sync.dma_start(
                                t[:], xp_d[c * 128:(c + 1) * 128, lo:lo + XN])
                        if last_x is not None:
                            _add_dep_helper(xd.ins, last_x.ins, sync=False,
                                            reason="xp stream order")
                        last_x = xd
                        xq[(b, c, qs)] = t
                # p_nodes prefetch rides behind this super-iter's xp tiles
                pt = pnl.tile([PP, XN], bf16, tag=f"pn{qs}", name=f"pn_{qs}")
                pd = nc.sync.dma_start(pt[:], pn_d[:, qs * XN:(qs + 1) * XN])
                _add_dep_helper(pd.ins, last_x.ins, sync=False,
                                reason="pn after xp of this super")
                pnt[qs] = pt
                if qs == 0:
                    pn0 = p0l.tile([128, 1280], bf16, tag="pn0")
                    d1 = nc.sync.dma_start(pn0[:], pn0_d[:])
                    hn0 = p0l.tile([128, 1280], bf16, tag="hn0")
                    d2 = nc.sync.dma_start(hn0[:], hn0_d[:])
                    _add_dep_helper(d1.ins, pd.ins, sync=False,
                                    reason="p0 after pn0 prefetch")
                    _add_dep_helper(d2.ins, d1.ins, sync=False,
                                    reason="p0 order")

            for qs in range(QS):
                for s in range(WPS):
                    k = qs * WPS + s
                    cs = slice(s * NW, (s + 1) * NW)
                    z = zp.tile([PP, NW], f32, tag="z", name=f"z_{k}")
                    pe_anchor(z, wp)
                    for h in range(2):
                        hs_ = slice(s * NW + h * 512, s * NW + (h + 1) * 512)
                        zs = slice(h * 512, (h + 1) * 512)
                        for c in range(2):
                            for b in range(B):
                                nc.tensor.matmul(
                                    z[b * MP:(b + 1) * MP, zs],
                                    wt[c], xq[(b, c, qs)][:, hs_],
                                    start=(c == 0), stop=(c == 1))
                    if k == NWT - 1:
                        asb = asb7
                    else:
                        ap_ = app.tile([PP, NW], f32, tag="att",
                                       name=f"att_{k}")
                        pe_anchor(ap_, wp)
                        for h in range(2):
                            gs = slice(qs * XN + s * NW + h * 512,
                                       qs * XN + s * NW + (h + 1) * 512)
                            nc.tensor.matmul(ap_[:, h * 512:(h + 1) * 512],
                                             l4t[:], att4t[:, gs],
                                             start=True, stop=True)
                        asb = attsb.tile([PP, NW], bf16, tag="asb",
                                         name=f"asb_{k}")
                        if k % 2 == 0:
                            nc.scalar.copy(asb[:], ap_[:])
                        else:
                            nc.vector.tensor_copy(asb[:], ap_[:])
                    sq = sqs.tile([PP, NW], bf16, tag="sq", name=f"sq_{k}")
                    if k == NWT - 1:
                        # last window at 512 granularity: halves the serial
                        # tail chain (stt -> square) gating the stats trigger
                        for h in range(2):
                            hw = slice(h * 512, (h + 1) * 512)
                            ysh = slice(qs * XN + s * NW + h * 512,
                                        qs * XN + s * NW + (h + 1) * 512)
                            nc.vector.scalar_tensor_tensor(
                                out=y_full[:, ysh], in0=z[:, hw], scalar=1.0,
                                in1=asb[:, hw], op0=Alu.mult, op1=Alu.mult,
                                accum_out=s1t[:, k + h:k + h + 1])
                            nc.scalar.activation(
                                sq[:, hw], y_full[:, ysh], Act.Square,
                                accum_out=s2t[:, k + h:k + h + 1])
                    else:
                        ys = slice(qs * XN + s * NW, qs * XN + (s + 1) * NW)
                        nc.vector.scalar_tensor_tensor(
                            out=y_full[:, ys], in0=z[:], scalar=1.0,
                            in1=asb[:], op0=Alu.mult, op1=Alu.mult,
                            accum_out=s1t[:, k:k + 1])
                        nc.scalar.activation(sq[:], y_full[:, ys], Act.Square,
                                             accum_out=s2t[:, k:k + 1])

            # ---- background-node path (independent of BN) ----
            o0 = p0l.tile([128, 1280], bf16, tag="o0")
            nc.gpsimd.tensor_add(o0[:], pn0[:], hn0[:])
            nc.scalar.dma_start(out0_d[:], o0[:])

            # ---- stats: local reduce, exchange, fold ----
            st4 = sm.tile([PP, 2], f32, tag="st4")
            nc.vector.reduce_sum(st4[:, 0:1], s1t[:], axis=mybir.AxisListType.X)
            nc.vector.reduce_sum(st4[:, 1:2], s2t[:], axis=mybir.AxisListType.X)
            # AllReduce my [128, 2] (sum, sumsq); the cross-batch channel
            # fold needs st[p] + st[(p+64)%128], done by reading the result
            # back twice - once straight, once with batch halves swapped
            # (three parallel readback DMAs on two HWDGE queues).
            cc_in = dr.tile([PP, 2], f32)
            cc_out = dr.tile([PP, 2], f32)
            nc.sync.dma_start(cc_in[:], st4[:])
            nc.gpsimd.collective_compute(
                "AllReduce", Alu.add,
                replica_groups=[list(range(N_CORES))],
                ins=[cc_in[:].opt()],
                outs=[cc_out[:].opt()],
            )
            arr = sm.tile([PP, 4], f32, tag="ar")
            nc.sync.dma_start(arr[:, 0:2], cc_out[:])
            nc.scalar.dma_start(arr[0:64, 2:4], cc_out[64:128, :])
            nc.sync.dma_start(arr[64:128, 2:4], cc_out[0:64, :])
            tot = sm.tile([PP, 2], f32, tag="tot")
            nc.vector.tensor_add(tot[:], arr[:, 0:2], arr[:, 2:4])

            # BN fold, all small per-partition DVE/ACT ops
            mE = sm.tile([PP, 2], f32, tag="mE")
            nc.vector.tensor_scalar_mul(mE[:], tot[:], 1.0 / NTOT)
            msq = sm.tile([PP, 1], f32, tag="msq")
            nc.vector.tensor_mul(msq[:], mE[:, 0:1], mE[:, 0:1])
            vpe = sm.tile([PP, 1], f32, tag="vpe")
            nc.vector.scalar_tensor_tensor(
                out=vpe[:], in0=mE[:, 1:2], scalar=EPS, in1=msq[:],
                op0=Alu.add, op1=Alu.subtract)
            sd = sm.tile([PP, 1], f32, tag="sd")
            nc.scalar.activation(sd[:], vpe[:], Act.Sqrt)
            r = sm.tile([PP, 1], f32, tag="r")
            nc.vector.reciprocal(r[:], sd[:])
            stb = sm.tile([PP, 2], f32, tag="stb")
            nc.vector.tensor_mul(stb[:, 0:1], r[:], gam)
            ms = sm.tile([PP, 1], f32, tag="ms")
            nc.vector.tensor_mul(ms[:], mE[:, 0:1], stb[:, 0:1])
            nc.vector.tensor_sub(stb[:, 1:2], bet, ms[:])

            # ---- phase 3: d = relu(s*y + t) ; out = d + pn ----
            # relu-affine on ACT (1 op) for most tiles, DVE 2-op path for the
            # rest.  GpSimd tensor ops are ~3.5x slower than DVE and contend
            # with DVE for SBUF ports - keep it out of the bulk work.
            ACT_TILES = {0, 2, 4, 5, 6, 7}
            for k in range(NWT):
                ys = slice(k * NW, (k + 1) * NW)
                ps = slice((k % WPS) * NW, (k % WPS + 1) * NW)
                pslice = pnt[k // WPS][:, ps]
                d = dbuf.tile([PP, NW], bf16, tag="d", name=f"d_{k}")
                if k in ACT_TILES:
                    nc.scalar.activation(d[:], y_full[:, ys], Act.Relu,
                                         scale=stb[:, 0:1], bias=stb[:, 1:2])
                    nc.vector.tensor_add(out_full[:, ys], d[:], pslice)
                else:
                    nc.vector.tensor_scalar(
                        d[:], y_full[:, ys], stb[:, 0:1], stb[:, 1:2],
                        op0=Alu.mult, op1=Alu.add)
                    nc.vector.scalar_tensor_tensor(
                        out=out_full[:, ys], in0=d[:], scalar=0.0,
                        in1=pslice, op0=Alu.max, op1=Alu.add)
                if k == 5:
                    nc.sync.dma_start(out_d[:, 4 * NW:6 * NW],
                                      out_full[:, 4 * NW:6 * NW])
                elif k in (1, 3):
                    os_ = slice((k - 1) * NW, (k + 1) * NW)
                    nc.sync.dma_start(out_d[:, os_], out_full[:, os_])
                elif k >= 6:
                    nc.sync.dma_start(out_d[:, ys], out_full[:, ys])

    # hoist excess sync waits onto same-engine NOPs (walrus wait-slot limits)
    SI = bass_rust.SyncInfo
    kk = 0
    for fn in nc.m.functions:
        for bb in fn.blocks:
            out = []
            for ins in bb.instructions:
                si = ins.sync_info
                if si is not None and len(si.on_wait) > 1:
                    waits = list(si.on_wait)
                    extra, keep = waits[:-1], waits[-1:]
                    for wti in extra:
                        nop = bass_rust.InstNoOp(name=f"Wsplit-{kk}", ins=[],
                                                 outs=[])
                        kk += 1
                        nop.engine = ins.engine
                        nop.sync_info = SI(on_wait=[wti], on_update=[])
                        out.append(nop)
                    ins.sync_info = SI(on_wait=keep, on_update=list(si.on_update))
                out.append(ins)
            bb.instructions = out
    return nc


def _get_nc():
    global _built
    if _built is None:
        _built = _build()
    return _built


def _make_consts(Wu, Wl, gamma_u, beta_u, gamma_l, beta_l):
    f32 = np.float32
    Wcat = np.concatenate([Wu, Wl], 0)                # [60, 256]
    lhsT = np.zeros((C, MP), f32)
    lhsT[:, 0:M] = Wcat.T
    wpack = np.zeros((128, 512), BF16)
    wpack[:, 0:MP] = lhsT[0:128].astype(BF16)
    wpack[:, MP:2 * MP] = lhsT[128:256].astype(BF16)
    l4 = np.zeros((4, 128), f32)
    l4[0, 0:40] = 1.0                                 # u channels <- h_att[1]
    l4[1, 40:60] = 1.0                                # l channels <- h_att[2]
    l4[2, MP:MP + 40] = 1.0
    l4[3, MP + 40:MP + 60] = 1.0
    cpk = np.zeros((128, 2), f32)
    gcat = 0.5 * np.concatenate([gamma_u, gamma_l])
    bcat = 0.5 * np.concatenate([beta_u, beta_l])
    cpk[0:M, 0] = gcat
    cpk[0:M, 1] = bcat
    cpk[MP:MP + M, 0] = gcat
    cpk[MP:MP + M, 1] = bcat
    return {"wpack": wpack, "l4": l4.astype(BF16), "cpk": cpk}


def _run(inputs, trace=False, trace_cores=None):
    from concourse import bass_utils
    p_nodes = np.asarray(inputs["p_nodes"], np.float32)
    h_nodes = np.asarray(inputs["h_nodes"], np.float32)
    xp = np.asarray(inputs["xp"], np.float32)
    h_att = np.asarray(inputs["h_att"], np.float32)
    const = _make_consts(np.asarray(inputs["Wu"], np.float32),
                         np.asarray(inputs["Wl"], np.float32),
                         np.asarray(inputs["gamma_u"], np.float32),
                         np.asarray(inputs["beta_u"], np.float32),
                         np.asarray(inputs["gamma_l"], np.float32),
                         np.asarray(inputs["beta_l"], np.float32))
    # host-side casts / pre-scales (layout only otherwise)
    xp16 = np.ascontiguousarray(xp.transpose(1, 0, 2, 3)).astype(BF16)
    # att rows: (h1 b0, h2 b0, h1 b1, h2 b1) -> [4, H, W] -> slice rows later
    att = np.stack([h_att[1, 0, 0], h_att[2, 0, 0],
                    h_att[1, 1, 0], h_att[2, 1, 0]], 0)
    att16 = att.reshape(4, 1, H, W).astype(BF16)
    pn16 = (0.5 * p_nodes[1:7]).astype(BF16)          # [6, B, HID, H, W]
    pn0_16 = (0.5 * p_nodes[0]).astype(BF16)          # [B, HID, H, W]
    hn0_16 = (0.5 * h_nodes[0]).astype(BF16)
    in_maps = []
    for i in range(N_CORES):
        hs = i * HS
        m = {
            "xp": np.ascontiguousarray(
                xp16[:, :, hs:hs + HS, :]).reshape(C, B * SPB),
            "att4": np.ascontiguousarray(
                att16[:, 0, hs:hs + HS, :]).reshape(4, SPB),
            "pn0": np.ascontiguousarray(
                pn0_16[:, :, hs:hs + HS, :]).reshape(128, 1280),
            "hn0": np.ascontiguousarray(
                hn0_16[:, :, hs:hs + HS, :]).reshape(128, 1280),
        }
        pn = np.zeros((PP, SPB), BF16)
        blk = pn16[:, :, :, hs:hs + HS, :].transpose(1, 0, 2, 3, 4)
        blk = blk.reshape(B, M, SPB)
        pn[0:M] = blk[0]
        pn[MP:MP + M] = blk[1]
        m["pn"] = pn
        m.update(const)
        in_maps.append(m)

    nc = _get_nc()
    res = bass_utils.run_bass_kernel_spmd(
        nc, in_maps, core_ids=list(range(N_CORES)), trace=trace,
        trace_cores=trace_cores)

    p_new = np.empty((7, B, HID, H, W), np.float32)
    for i in range(N_CORES):
        hs = i * HS
        om = res.results[i]["out_main"].astype(np.float32)   # [128, SPB]
        o0 = res.results[i]["out0"].astype(np.float32)       # [128, 1280]
        p_new[0, :, :, hs:hs + HS, :] = o0.reshape(B, HID, HS, W)
        for b in range(B):
            blk = om[b * MP:b * MP + M].reshape(6, HID, HS, W)
            p_new[1:7, b, :, hs:hs + HS, :] = blk
    return p_new, res


def kernel(**inputs) -> np.ndarray:
    return _run(inputs, trace=False)[0]


# revision 22
# speedup vs baseline: 1.4803x; 1.4803x over previous
"""Trainium2 Bass kernel for nn_GNN_82781199663565 (gnn_message_passing).

Computation (see reference):
  du = relu(BN(einsum(h_att[1]*xp, Wu)))   # [B, 40, H, W]
  dl = relu(BN(einsum(h_att[2]*xp, Wl)))   # [B, 20, H, W]
  p_new[0]   = 0.5*(h_nodes[0] + p_nodes[0])
  p_new[1:5] = 0.5*(p_nodes[1:5] + du4)    # du reshaped to [4, B, 10, H, W]
  p_new[5:7] = 0.5*(p_nodes[5:7] + dl2)
(f_nodes, h_att[0], h_nodes[1:] are unused.)

Strategy vs the fp32 baseline (155us):
 - All bulk tensors cast to bf16 on host: xp stream 16.8->8.4 MB/core,
   p_nodes prefetch + output store halved.  Matmuls run bf16 (4x PE rate).
 - The [128, n] replicated attention operand is no longer streamed from
   HBM (4.2 MB/core); instead the compact [4, n] h_att rows are broadcast
   across partitions on the PE via a 0/1 selector matmul (L4), and applied
   to z from PSUM.
 - Sync-BN stats exchange: instead of the ncfw AllReduce (~28us end to
   end) each core writes its [128,4] partial-sums tile directly into the
   SBUF of all 7 peers with XOR-relative remote_dma_broadcast (slot j of
   my gather buffer is written by core me^j), then reduces locally on DVE.
 - Cross-batch stat folding uses a swapped-halves copy of the stats made
   with one SBUF->SBUF DMA, so the entire BN fold is small DVE ops - no
   tensor-engine matmuls on the post-collective critical path.
 - p_nodes are pre-scaled by 0.5 on the host, so phase 3 is
   d = relu(s*y + t) (ACT) ; out = d + pn (DVE), tiles split across
   engines.
"""
import sys
sys.path.insert(0, '/opt/trn_rl_repo')

import numpy as np
import ml_dtypes

BF16 = np.dtype(ml_dtypes.bfloat16)

N_CORES = 8
B, C, HID, H, W = 2, 256, 10, 256, 256
EPS = 1e-5
HS = H // N_CORES            # 32 H-rows per core
SPB = HS * W                 # spatial elems per batch image per core: 8192
M = 60                       # real output channels (40 u + 20 l)
MP = 64                      # padded to 64 -> groups tile partitions exactly
PP = 128
NW = 1024                    # window columns for vector/scalar ops
XN = 4096                    # xp super-tile columns (1 MiB bf16 DMAs)
QS = SPB // XN               # 2 super-iterations
WPS = XN // NW               # 4 windows per super-iteration
NWT = SPB // NW              # 8 windows total
NTOT = float(B * H * W)      # BN stat count: 131072

# NOTE: a direct remote_dma_broadcast stats exchange (XOR-relative SPMD
# all-gather, ~4us) was designed and built, but this container's walrus
# compiler rejects the remote-DMA ucode ISA ("ISA wrong length"), so the
# ncfw AllReduce (~35-45us exposed: entry barrier + trigger poll + mesh)
# is the only available cross-core primitive.

_built = None


def _build():
    import concourse.bass as bass
    import concourse.tile as tile
    from concourse import mybir
    from concourse.bass import _add_dep_helper
    import bass_rust

    f32 = mybir.dt.float32
    bf16 = mybir.dt.bfloat16
    Alu = mybir.AluOpType
    Act = mybir.ActivationFunctionType

    nc = bass.Bass("TRN2", target_bir_lowering=False, debug=False,
                   num_devices=N_CORES)

    xp_d = nc.dram_tensor("xp", [C, B * SPB], bf16, kind="ExternalInput").ap()
    att4_d = nc.dram_tensor("att4", [4, SPB], bf16, kind="ExternalInput").ap()
    pn_d = nc.dram_tensor("pn", [PP, SPB], bf16, kind="ExternalInput").ap()
    pn0_d = nc.dram_tensor("pn0", [128, 1280], bf16, kind="ExternalInput").ap()
    hn0_d = nc.dram_tensor("hn0", [128, 1280], bf16, kind="ExternalInput").ap()
    wp_d = nc.dram_tensor("wpack", [128, 512], bf16, kind="ExternalInput").ap()
    l4_d = nc.dram_tensor("l4", [4, 128], bf16, kind="ExternalInput").ap()
    cpk_d = nc.dram_tensor("cpk", [128, 2], f32, kind="ExternalInput").ap()

    out_d = nc.dram_tensor("out_main", [PP, SPB], bf16,
                           kind="ExternalOutput").ap()
    out0_d = nc.dram_tensor("out0", [128, 1280], bf16,
                            kind="ExternalOutput").ap()

    def pe_anchor(psum_tile, wp):
        # tiny matmul writing one psum element: absorbs the psum slot-release
        # wait so real matmuls carry <=1 wait
        nc.tensor.matmul(psum_tile[0:1, 0:1], wp[0:1, 0:1], wp[0:1, 0:1],
                         start=True, stop=True, skip_group_check=True)

    with tile.TileContext(nc) as tc:
        with (
            tc.tile_pool(name="consts", bufs=1) as cpool,
            tc.tile_pool(name="attc", bufs=1) as attc,
            tc.tile_pool(name="xin", bufs=2) as xin,
            tc.tile_pool(name="attsb", bufs=3) as attsb,
            tc.tile_pool(name="ybuf", bufs=1) as ybuf,
            tc.tile_pool(name="sqs", bufs=2) as sqs,
            tc.tile_pool(name="pnl", bufs=1) as pnl,
            tc.tile_pool(name="p0l", bufs=1) as p0l,
            tc.tile_pool(name="sm", bufs=1) as sm,
            tc.tile_pool(name="dbuf", bufs=3) as dbuf,
            tc.tile_pool(name="zp", bufs=2, space="PSUM") as zp,
            tc.tile_pool(name="app", bufs=2, space="PSUM") as app,
            tc.tile_pool(name="dram", bufs=1, space="DRAM") as dr,
        ):
            wp = cpool.tile([128, 512], bf16)
            nc.sync.dma_start(wp[:], wp_d[:])
            l4t = cpool.tile([4, 128], bf16)
            nc.sync.dma_start(l4t[:], l4_d[:])
            cpk = cpool.tile([128, 2], f32)
            nc.sync.dma_start(cpk[:], cpk_d[:])
            gam = cpk[:, 0:1]                    # 0.5*gamma (z layout)
            bet = cpk[:, 1:2]                    # 0.5*beta
            wt = [wp[:, 0:MP], wp[:, MP:2 * MP]]

            att4t = attc.tile([4, SPB], bf16)
            nc.sync.dma_start(att4t[:], att4_d[:])

            y_full = ybuf.tile([PP, SPB], bf16, tag="y")
            out_full = ybuf.tile([PP, SPB], bf16, tag="o")
            s1t = sm.tile([PP, NWT + 1], f32, tag="s1t")
            s2t = sm.tile([PP, NWT + 1], f32, tag="s2t")

            # ---- PE warm-up: ~3us of dummy matmuls trips the HAM into
            # the 2.4 GHz state before the first xp tile lands ----
            wz = zp.tile([PP, NW], f32, tag="z", name="warm_z")
            for _ in range(8):
                nc.tensor.matmul(wz[0:MP, 0:384], wp[:, 0:MP], wp[:, 128:512],
                                 start=True, stop=True, skip_group_check=True)

            # broadcast the LAST window's attention early: removes the PSUM
            # copy from the post-stream serial tail that gates the stats
            # trigger
            ap7 = app.tile([PP, NW], f32, tag="att", name="att_pre7")
            pe_anchor(ap7, wp)
            for h in range(2):
                gs = slice(SPB - NW + h * 512, SPB - NW + (h + 1) * 512)
                nc.tensor.matmul(ap7[:, h * 512:(h + 1) * 512],
                                 l4t[:], att4t[:, gs], start=True, stop=True)
            asb7 = attsb.tile([PP, NW], bf16, tag="asb7", bufs=1, name="asb7")
            nc.scalar.copy(asb7[:], ap7[:])

            # ---- phase 1: stream xp, matmul, att broadcast, y = z*a ----
            xq = {}
            pnt = {}
            last_x = None
            for qs in range(QS):
                for b in range(B):
                    for c in range(2):
                        t = xin.tile([128, XN], bf16, tag=f"x{b}{c}",
                                     name=f"x{b}{c}_{qs}")
                        lo = b * SPB + qs * XN
                        if qs == 0 and b == 0 and c == 0:
                            # split first load: matmuls start on the first
                            # half while the second half streams
                            nc.sync.dma_start(
                                t[:, 0:XN // 2],
                                xp_d[c * 128:(c + 1) * 128, lo:lo + XN // 2])
                            xd = nc.sync.dma_start(
                                t[:, XN // 2:XN],
                                xp_d[c * 128:(c + 1) * 128,
                                     lo + XN // 2:lo + XN])
                        else:
                            xd = nc.sync.dma_start(
                                t[:], xp_d[c * 128:(c + 1) * 128, lo:lo + XN])
                        if last_x is not None:
                            _add_dep_helper(xd.ins, last_x.ins, sync=False,
                                            reason="xp stream order")
                        last_x = xd
                        xq[(b, c, qs)] = t
                # p_nodes prefetch rides behind this super-iter's xp tiles
                pt = pnl.tile([PP, XN], bf16, tag=f"pn{qs}", name=f"pn_{qs}")
                pd = nc.sync.dma_start(pt[:], pn_d[:, qs * XN:(qs + 1) * XN])
                _add_dep_helper(pd.ins, last_x.ins, sync=False,
                                reason="pn after xp of this super")
                pnt[qs] = pt
                if qs == 0:
                    pn0 = p0l.tile([128, 1280], bf16, tag="pn0")
                    d1 = nc.sync.dma_start(pn0[:], pn0_d[:])
                    hn0 = p0l.tile([128, 1280], bf16, tag="hn0")
                    d2 = nc.sync.dma_start(hn0[:], hn0_d[:])
                    _add_dep_helper(d1.ins, pd.ins, sync=False,
                                    reason="p0 after pn0 prefetch")
                    _add_dep_helper(d2.ins, d1.ins, sync=False,
                                    reason="p0 order")

            for qs in range(QS):
                for s in range(WPS):
                    k = qs * WPS + s
                    cs = slice(s * NW, (s + 1) * NW)
                    z = zp.tile([PP, NW], f32, tag="z", name=f"z_{k}")
                    pe_anchor(z, wp)
                    for h in range(2):
                        hs_ = slice(s * NW + h * 512, s * NW + (h + 1) * 512)
                        zs = slice(h * 512, (h + 1) * 512)
                        for c in range(2):
                            for b in range(B):
                                nc.tensor.matmul(
                                    z[b * MP:(b + 1) * MP, zs],
                                    wt[c], xq[(b, c, qs)][:, hs_],
                                    start=(c == 0), stop=(c == 1))
                    if k == NWT - 1:
                        asb = asb7
                    else:
                        ap_ = app.tile([PP, NW], f32, tag="att",
                                       name=f"att_{k}")
                        pe_anchor(ap_, wp)
                        for h in range(2):
                            gs = slice(qs * XN + s * NW + h * 512,
                                       qs * XN + s * NW + (h + 1) * 512)
                            nc.tensor.matmul(ap_[:, h * 512:(h + 1) * 512],
                                             l4t[:], att4t[:, gs],
                                             start=True, stop=True)
                        asb = attsb.tile([PP, NW], bf16, tag="asb",
                                         name=f"asb_{k}")
                        if k % 2 == 0:
                            nc.scalar.copy(asb[:], ap_[:])
                        else:
                            nc.vector.tensor_copy(asb[:], ap_[:])
                    sq = sqs.tile([PP, NW], bf16, tag="sq", name=f"sq_{k}")
                    if k == NWT - 1:
                        # last window at 512 granularity: halves the serial
                        # tail chain (stt -> square) gating the stats trigger
                        for h in range(2):
                            hw = slice(h * 512, (h + 1) * 512)
                            ysh = slice(qs * XN + s * NW + h * 512,
                                        qs * XN + s * NW + (h + 1) * 512)
                            nc.vector.scalar_tensor_tensor(
                                out=y_full[:, ysh], in0=z[:, hw], scalar=1.0,
                                in1=asb[:, hw], op0=Alu.mult, op1=Alu.mult,
                                accum_out=s1t[:, k + h:k + h + 1])
                            nc.scalar.activation(
                                sq[:, hw], y_full[:, ysh], Act.Square,
                                accum_out=s2t[:, k + h:k + h + 1])
                    else:
                        ys = slice(qs * XN + s * NW, qs * XN + (s + 1) * NW)
                        nc.vector.scalar_tensor_tensor(
                            out=y_full[:, ys], in0=z[:], scalar=1.0,
                            in1=asb[:], op0=Alu.mult, op1=Alu.mult,
                            accum_out=s1t[:, k:k + 1])
                        nc.scalar.activation(sq[:], y_full[:, ys], Act.Square,
                                             accum_out=s2t[:, k:k + 1])

            # ---- background-node path (independent of BN) ----
            o0 = p0l.tile([128, 1280], bf16, tag="o0")
            nc.gpsimd.tensor_add(o0[:], pn0[:], hn0[:])
            nc.scalar.dma_start(out0_d[:], o0[:])

            # ---- stats: local reduce, exchange, fold ----
            st4 = sm.tile([PP, 2], f32, tag="st4")
            nc.vector.reduce_sum(st4[:, 0:1], s1t[:], axis=mybir.AxisListType.X)
            nc.vector.reduce_sum(st4[:, 1:2], s2t[:], axis=mybir.AxisListType.X)
            # AllReduce my [128, 2] (sum, sumsq); the cross-batch channel
            # fold needs st[p] + st[(p+64)%128], done by reading the result
            # back twice - once straight, once with batch halves swapped
            # (three parallel readback DMAs on two HWDGE queues).
            cc_in = dr.tile([PP, 2], f32)
            cc_out = dr.tile([PP, 2], f32)
            nc.sync.dma_start(cc_in[:], st4[:])
            nc.gpsimd.collective_compute(
                "AllReduce", Alu.add,
                replica_groups=[list(range(N_CORES))],
                ins=[cc_in[:].opt()],
                outs=[cc_out[:].opt()],
            )
            arr = sm.tile([PP, 4], f32, tag="ar")
            nc.sync.dma_start(arr[:, 0:2], cc_out[:])
            nc.scalar.dma_start(arr[0:64, 2:4], cc_out[64:128, :])
            nc.sync.dma_start(arr[64:128, 2:4], cc_out[0:64, :])
            tot = sm.tile([PP, 2], f32, tag="tot")
            nc.vector.tensor_add(tot[:], arr[:, 0:2], arr[:, 2:4])

            # BN fold, all small per-partition DVE/ACT ops
            mE = sm.tile([PP, 2], f32, tag="mE")
            nc.vector.tensor_scalar_mul(mE[:], tot[:], 1.0 / NTOT)
            msq = sm.tile([PP, 1], f32, tag="msq")
            nc.vector.tensor_mul(msq[:], mE[:, 0:1], mE[:, 0:1])
            vpe = sm.tile([PP, 1], f32, tag="vpe")
            nc.vector.scalar_tensor_tensor(
                out=vpe[:], in0=mE[:, 1:2], scalar=EPS, in1=msq[:],
                op0=Alu.add, op1=Alu.subtract)
            sd = sm.tile([PP, 1], f32, tag="sd")
            nc.scalar.activation(sd[:], vpe[:], Act.Sqrt)
            r = sm.tile([PP, 1], f32, tag="r")
            nc.vector.reciprocal(r[:], sd[:])
            stb = sm.tile([PP, 2], f32, tag="stb")
            nc.vector.tensor_mul(stb[:, 0:1], r[:], gam)
            ms = sm.tile([PP, 1], f32, tag="ms")
            nc.vector.tensor_mul(ms[:], mE[:, 0:1], stb[:, 0:1])
            nc.vector.tensor_sub(stb[:, 1:2], bet, ms[:])

            # ---- phase 3: d = relu(s*y + t) ; out = d + pn ----
            # relu-affine on ACT (1 op) for most tiles, DVE 2-op path for the
            # rest.  GpSimd tensor ops are ~3.5x slower than DVE and contend
            # with DVE for SBUF ports - keep it out of the bulk work.
            ACT_TILES = {0, 2, 4, 5, 6, 7}
            for k in range(NWT):
                ys = slice(k * NW, (k + 1) * NW)
                ps = slice((k % WPS) * NW, (k % WPS + 1) * NW)
                pslice = pnt[k // WPS][:, ps]
                d = dbuf.tile([PP, NW], bf16, tag="d", name=f"d_{k}")
                if k in ACT_TILES:
                    nc.scalar.activation(d[:], y_full[:, ys], Act.Relu,
                                         scale=stb[:, 0:1], bias=stb[:, 1:2])
                    nc.vector.tensor_add(out_full[:, ys], d[:], pslice)
                else:
                    nc.vector.tensor_scalar(
                        d[:], y_full[:, ys], stb[:, 0:1], stb[:, 1:2],
                        op0=Alu.mult, op1=Alu.add)
                    nc.vector.scalar_tensor_tensor(
                        out=out_full[:, ys], in0=d[:], scalar=0.0,
                        in1=pslice, op0=Alu.max, op1=Alu.add)
                if k == 5:
                    nc.sync.dma_start(out_d[:, 4 * NW:6 * NW],
                                      out_full[:, 4 * NW:6 * NW])
                elif k in (1, 3):
                    os_ = slice((k - 1) * NW, (k + 1) * NW)
                    nc.sync.dma_start(out_d[:, os_], out_full[:, os_])
                elif k >= 6:
                    nc.sync.dma_start(out_d[:, ys], out_full[:, ys])

    # hoist excess sync waits onto same-engine NOPs (walrus wait-slot limits)
    SI = bass_rust.SyncInfo
    kk = 0
    for fn in nc.m.functions:
        for bb in fn.blocks:
            out = []
            for ins in bb.instructions:
                si = ins.sync_info
                if si is not None and len(si.on_wait) > 1:
                    waits = list(si.on_wait)
                    extra, keep = waits[:-1], waits[-1:]
                    for wti in extra:
                        nop = bass_rust.InstNoOp(name=f"Wsplit-{kk}", ins=[],
                                                 outs=[])
                        kk += 1
                        nop.engine = ins.engine
                        nop.sync_info = SI(on_wait=[wti], on_update=[])
                        out.append(nop)
                    ins.sync_info = SI(on_wait=keep, on_update=list(si.on_update))
                out.append(ins)
            bb.instructions = out
    return nc


def _get_nc():
    global _built
    if _built is None:
        _built = _build()
    return _built


def _make_consts(Wu, Wl, gamma_u, beta_u, gamma_l, beta_l):
    f32 = np.float32
    Wcat = np.concatenate([Wu, Wl], 0)                # [60, 256]
    lhsT = np.zeros((C, MP), f32)
    lhsT[:, 0:M] = Wcat.T
    wpack = np.zeros((128, 512), BF16)
    wpack[:, 0:MP] = lhsT[0:128].astype(BF16)
    wpack[:, MP:2 * MP] = lhsT[128:256].astype(BF16)
    l4 = np.zeros((4, 128), f32)
    l4[0, 0:40] = 1.0                                 # u channels <- h_att[1]
    l4[1, 40:60] = 1.0                                # l channels <- h_att[2]
    l4[2, MP:MP + 40] = 1.0
    l4[3, MP + 40:MP + 60] = 1.0
    cpk = np.zeros((128, 2), f32)
    gcat = 0.5 * np.concatenate([gamma_u, gamma_l])
    bcat = 0.5 * np.concatenate([beta_u, beta_l])
    cpk[0:M, 0] = gcat
    cpk[0:M, 1] = bcat
    cpk[MP:MP + M, 0] = gcat
    cpk[MP:MP + M, 1] = bcat
    return {"wpack": wpack, "l4": l4.astype(BF16), "cpk": cpk}


def _run(inputs, trace=False, trace_cores=None):
    from concourse import bass_utils
    p_nodes = np.asarray(inputs["p_nodes"], np.float32)
    h_nodes = np.asarray(inputs["h_nodes"], np.float32)
    xp = np.asarray(inputs["xp"], np.float32)
    h_att = np.asarray(inputs["h_att"], np.float32)
    const = _make_consts(np.asarray(inputs["Wu"], np.float32),
                         np.asarray(inputs["Wl"], np.float32),
                         np.asarray(inputs["gamma_u"], np.float32),
                         np.asarray(inputs["beta_u"], np.float32),
                         np.asarray(inputs["gamma_l"], np.float32),
                         np.asarray(inputs["beta_l"], np.float32))
    # host-side casts / pre-scales (layout only otherwise)
    xp16 = np.ascontiguousarray(xp.transpose(1, 0, 2, 3)).astype(BF16)
    # att rows: (h1 b0, h2 b0, h1 b1, h2 b1) -> [4, H, W] -> slice rows later
    att = np.stack([h_att[1, 0, 0], h_att[2, 0, 0],
                    h_att[1, 1, 0], h_att[2, 1, 0]], 0)
    att16 = att.reshape(4, 1, H, W).astype(BF16)
    pn16 = (0.5 * p_nodes[1:7]).astype(BF16)          # [6, B, HID, H, W]
    pn0_16 = (0.5 * p_nodes[0]).astype(BF16)          # [B, HID, H, W]
    hn0_16 = (0.5 * h_nodes[0]).astype(BF16)
    in_maps = []
    for i in range(N_CORES):
        hs = i * HS
        m = {
            "xp": np.ascontiguousarray(
                xp16[:, :, hs:hs + HS, :]).reshape(C, B * SPB),
            "att4": np.ascontiguousarray(
                att16[:, 0, hs:hs + HS, :]).reshape(4, SPB),
            "pn0": np.ascontiguousarray(
                pn0_16[:, :, hs:hs + HS, :]).reshape(128, 1280),
            "hn0": np.ascontiguousarray(
                hn0_16[:, :, hs:hs + HS, :]).reshape(128, 1280),
        }
        pn = np.zeros((PP, SPB), BF16)
        blk = pn16[:, :, :, hs:hs + HS, :].transpose(1, 0, 2, 3, 4)
        blk = blk.reshape(B, M, SPB)
        pn[0:M] = blk[0]
        pn[MP:MP + M] = blk[1]
        m["pn"] = pn
        m.update(const)
        in_maps.append(m)

    nc = _get_nc()
    res = bass_utils.run_bass_kernel_spmd(
        nc, in_maps, core_ids=list(range(N_CORES)), trace=trace,
        trace_cores=trace_cores)

    p_new = np.empty((7, B, HID, H, W), np.float32)
    for i in range(N_CORES):
        hs = i * HS
        om = res.results[i]["out_main"].astype(np.float32)   # [128, SPB]
        o0 = res.results[i]["out0"].astype(np.float32)       # [128, 1280]
        p_new[0, :, :, hs:hs + HS, :] = o0.reshape(B, HID, HS, W)
        for b in range(B):
            blk = om[b * MP:b * MP + M].reshape(6, HID, HS, W)
            p_new[1:7, b, :, hs:hs + HS, :] = blk
    return p_new, res


def kernel(**inputs) -> np.ndarray:
    return _run(inputs, trace=False)[0]
